# revision 47
# baseline (speedup 1.0000x reference)
"""Multi-head attention (B=2, S=2048, D=1024, H=16) on 8 Trainium2 NeuronCores.

Sharding: core c handles batch b = c//4 and head-group g = c%4 (4 heads,
a 256-wide column slice of wq/wk/wv and row slice of wo).  Each core
computes a full [S, D] partial of the output projection; the host sums
the 4 partials per batch and adds the output bias.

Per-core kernel, tuned around the PE p-state rule (the PE only reaches
2.4 GHz after 3us of gap-free execution, and any stall resets it):
  - scores run THREE chunks ahead of the ctx matmuls (lag-3) so the
    ~1.25us probs latency fits inside the in-order PE's window; all 8
    PSUM banks go to a 3-deep scores ring (6) + ctx accumulators (2).
    Out-projection, Q-projection and denominator-replicate matmuls
    allocate transient slots from the same scores ring.
  - probs: 11 of 16 key-chunks exp on the scalar engine (which does
    NOTHING else, so exps are never queue-delayed); 5 chunks on the
    vector engine via a Schraudolph bit-trick (s*log2e*128 +
    (127-sigma)*128 rounded to int16 IS the bf16 encoding of ~exp(s)).
  - Q-projection for query chunks 1-3 is deferred into the attention
    phase as PE filler units (shrinks the serial head by ~10us and
    plugs probs-latency windows).
  - normalize: denominator rows evacuated as f32r, replicated by a K=1
    matmul (filler unit), reciprocal_approx_fast on the replicated
    [64,512] tile (~1.6 cyc/elem vs 8 for reciprocal()), multiplies on
    the otherwise-idle GPSIMD.
  - out-projection: [128,1024] PSUM slots, 4 matmuls each, single DVE
    evacuation, 4KB-row DMA to DRAM.
  - input DMA: full-4KB-row descriptors, each tensor split across the
    sync/scalar/gpsimd DMA queues, K first (it gates the first
    projection), then V, then Q.
"""

import os
import sys

import ml_dtypes
import numpy as np

if "/opt/trn_rl_repo" not in sys.path:
    sys.path.insert(0, "/opt/trn_rl_repo")

B, S, D, H = 2, 2048, 1024, 16
DH = D // H  # 64
NCORES = 8
GC = 256  # column slice per core (4 heads)
NP = 2  # head pairs per core
KC = D // 128  # 8 contraction chunks
SQC = S // 512  # 4 query chunks
SKC = S // 128  # 16 key chunks
WARMUP = 66
LAG = 3
# key-chunks whose probs ride the DVE; 14+15 keep ACT free at the step
# seam so the ctx-PSUM evacuations never wait behind a trailing exp
DVE_CHUNKS = (2, 5, 8, 11, 14, 15)

# Schraudolph exp in bf16 bit-space: int16(round(s*log2e*128 + (127-sigma)*128))
# reinterpreted as bf16 approximates exp(s) to ~3%.  Softmax scale 1/8 folded in.
EXPA = 0.125 * 1.4426950408889634 * 128.0
EXPB = (127.0 - 0.0579) * 128.0

_CACHE = {}


def _build_program():
    import concourse.bass as bass
    import concourse.tile as tile
    from concourse import bacc, mybir

    F32 = mybir.dt.float32
    F32R = mybir.dt.float32r
    BF16 = mybir.dt.bfloat16
    I16 = mybir.dt.int16
    EXP = mybir.ActivationFunctionType.Exp
    ALU = mybir.AluOpType
    PSUM = bass.MemorySpace.PSUM

    nc = bacc.Bacc()

    qT = nc.dram_tensor("qT", (D, S), BF16, kind="ExternalInput").ap()
    kT = nc.dram_tensor("kT", (D, S), BF16, kind="ExternalInput").ap()
    vT = nc.dram_tensor("vT", (D, S), BF16, kind="ExternalInput").ap()
    # weights arrive host-packed as [128, KC*GC] (partition-major) so each
    # partition's DMA line is one contiguous 4KB row instead of 8x512B
    wqs = nc.dram_tensor("wqs", (128, KC * GC), BF16, kind="ExternalInput").ap()
    wks = nc.dram_tensor("wks", (128, KC * GC), BF16, kind="ExternalInput").ap()
    wvs = nc.dram_tensor("wvs", (128, KC * GC), BF16, kind="ExternalInput").ap()
    wos = nc.dram_tensor("wos", (GC, D), BF16, kind="ExternalInput").ap()
    bqs = nc.dram_tensor("bqs", (NP, 128, 1), F32, kind="ExternalInput").ap()
    bks = nc.dram_tensor("bks", (NP, 128, 1), F32, kind="ExternalInput").ap()
    bvs = nc.dram_tensor("bvs", (1, GC), F32R, kind="ExternalInput").ap()
    outp = nc.dram_tensor("outp", (S, D), F32, kind="ExternalOutput").ap()

    with tile.TileContext(nc) as tc:
        with (
            tc.tile_pool(name="const", bufs=1) as const,
            tc.tile_pool(name="raw", bufs=1) as rawp,
            tc.tile_pool(name="probs", bufs=6) as probs,
            tc.tile_pool(name="small", bufs=2) as small,
            tc.tile_pool(name="outsb", bufs=3) as outsb,
            tc.tile_pool(name="psc", bufs=3, space=PSUM) as psc,
            tc.tile_pool(name="pctx", bufs=1, space=PSUM) as pctx,
        ):
            # ---- constants / weights ----
            wq_t = const.tile([128, KC, GC], BF16, name="wqt", tag="wqt")
            wk_t = const.tile([128, KC, GC], BF16, name="wkt", tag="wkt")
            wv_t = const.tile([128, KC, GC], BF16, name="wvt", tag="wvt")
            wo_t = [const.tile([128, D], BF16, name=f"wo{m}", tag=f"wo{m}") for m in range(NP)]
            bq_t = [const.tile([128, 1], F32, name=f"bq{m}", tag=f"bq{m}") for m in range(NP)]
            bk_t = [const.tile([128, 1], F32, name=f"bk{m}", tag=f"bk{m}") for m in range(NP)]
            bv_row = const.tile([1, GC], F32R, name="bvrow", tag="bvrow")
            ones_t = const.tile([128, 128], F32R, name="ones", tag="ones")

            # PE warmup chain: keeps the HAM clock-gate at 8/8 through the
            # initial input DMA window so projections start at 2.4 GHz.
            wu = const.tile([128, 512], BF16, name="wu", tag="wu")
            nc.vector.memset(wu[:], 0.0)
            wup = psc.tile([128, 1024], F32, name="sc", tag="sc")
            for w in range(WARMUP):
                nc.tensor.matmul(
                    wup[:, 0:512], wu[:, 0:128], wu[:],
                    start=(w == 0), stop=(w == WARMUP - 1),
                )

            # Input DMA: K gates the first projection so it rides the two
            # fastest-starting queues; V next; Q last (its chunk-0 proj is
            # the last head-phase consumer, chunks 1-3 are deferred).
            q_raw = rawp.tile([128, KC, S], BF16, name="qraw", tag="qraw")
            k_raw = rawp.tile([128, KC, S], BF16, name="kraw", tag="kraw")
            v_raw = rawp.tile([128, KC, S], BF16, name="vraw", tag="vraw")
            qTr = qT.rearrange("(c p) s -> p c s", p=128)
            kTr = kT.rearrange("(c p) s -> p c s", p=128)
            vTr = vT.rearrange("(c p) s -> p c s", p=128)
            HC = KC // 2

            # strict K -> V -> Q order on the two bulk queues so K (which
            # gates the first projection) gets the full HBM bandwidth;
            # weights ride the gpsimd queue.
            nc.gpsimd.dma_start(wk_t[:], wks.rearrange("p (c g) -> p c g", c=KC))
            nc.sync.dma_start(k_raw[:, 0:HC, :], kTr[:, 0:HC, :])
            nc.scalar.dma_start(k_raw[:, HC:KC, :], kTr[:, HC:KC, :])
            nc.sync.dma_start(q_raw[:, 0:HC, :], qTr[:, 0:HC, :])
            nc.scalar.dma_start(q_raw[:, HC:KC, :], qTr[:, HC:KC, :])
            nc.sync.dma_start(v_raw[:, 0:HC, :], vTr[:, 0:HC, :])
            nc.scalar.dma_start(v_raw[:, HC:KC, :], vTr[:, HC:KC, :])
            for m in range(NP):
                nc.gpsimd.dma_start(bk_t[m][:], bks[m])
            nc.gpsimd.dma_start(wv_t[:], wvs.rearrange("p (c g) -> p c g", c=KC))
            nc.gpsimd.dma_start(bv_row[:], bvs[:])
            nc.gpsimd.dma_start(wq_t[:], wqs.rearrange("p (c g) -> p c g", c=KC))
            for m in range(NP):
                nc.gpsimd.dma_start(bq_t[m][:], bqs[m])
            for m in range(NP):
                nc.gpsimd.dma_start(wo_t[m][:], wos[m * 128 : (m + 1) * 128, :])

            ones_f = const.tile([128, 128], F32, name="onesf", tag="onesf")
            nc.vector.memset(ones_f[:], 1.0)
            nc.vector.tensor_copy(ones_t[:], ones_f[:])

            # bv broadcast to all partitions: [128, GC] = ones[1,128].T @ bv[1,GC]
            bvb = const.tile([128, GC], F32, name="bvb", tag="bvb")
            bvp = psc.tile([128, 1024], F32, name="sc", tag="sc")
            nc.tensor.matmul(
                bvp[:, :GC], ones_t[0:1, 0:128], bv_row[:],
                start=True, stop=True,
            )
            nc.vector.tensor_copy(bvb[:], bvp[:, :GC])

            # ---- persistent activation tiles ----
            QT = [const.tile([128, S], BF16, name=f"QT{m}", tag=f"QT{m}") for m in range(NP)]
            KT = [const.tile([128, S], BF16, name=f"KT{m}", tag=f"KT{m}") for m in range(NP)]
            VH = [const.tile([128, 4, 66], BF16, name=f"VH{i}", tag=f"VH{i}") for i in range(SKC)]
            ctxT = [const.tile([128, S], BF16, name=f"ctxT{m}", tag=f"ctxT{m}") for m in range(NP)]

            for i in range(SKC):
                nc.vector.memset(VH[i][:, :, 64:65], 1.0)

            # ---- phase 1: projections (K, V, then Q chunk 0 only) ----
            def proj_group(raw, w_t, b_t, dst, nq, m):
                ps = psc.tile([128, 1024], F32, name="sc", tag="sc")
                for k in range(KC):
                    nc.tensor.matmul(
                        ps[:, 0:512],
                        w_t[:, k, m * 128 : (m + 1) * 128],
                        raw[:, k, nq * 512 : (nq + 1) * 512],
                        start=(k == 0),
                        stop=(k == KC - 1),
                    )
                nc.vector.tensor_scalar_add(
                    dst[m][:, nq * 512 : (nq + 1) * 512], ps[:, 0:512], b_t[m][:]
                )

            for nq in range(SQC):
                for m in range(NP):
                    proj_group(k_raw, wk_t, bk_t, KT, nq, m)
            for m in range(NP):
                proj_group(q_raw, wq_t, bq_t, QT, 0, m)

            # deferred projection filler units: V chunks pop inside step 0
            # (each before its ctx deadline), Q chunks 1-3 across steps 0-1
            def vproj_unit(i):
                def emit():
                    ps = psc.tile([128, 1024], F32, name="sc", tag="sc")
                    for k in range(KC):
                        nc.tensor.matmul(
                            ps[:, :GC],
                            v_raw[:, k, i * 128 : (i + 1) * 128],
                            wv_t[:, k, :],
                            start=(k == 0),
                            stop=(k == KC - 1),
                        )
                    nc.vector.tensor_add(
                        VH[i][:, :, 0:64],
                        ps[:, :GC].rearrange("p (h d) -> p h d", h=4),
                        bvb[:].rearrange("p (h d) -> p h d", h=4),
                    )

                return emit

            def qproj_unit(nq, m):
                def emit():
                    proj_group(q_raw, wq_t, bq_t, QT, nq, m)

                return emit

            # ---- attention ----
            def attn_step(sq, m, pe_fill, norm_pieces, lead_pops=0, per_slot=1):
                ctA = pctx.tile([128, 512], F32, name="ctA", tag="ctA")
                ctB = pctx.tile([128, 512], F32, name="ctB", tag="ctB")
                pb = {}

                def scores(i):
                    sc = psc.tile([128, 1024], F32, name="sc", tag="sc")
                    nc.tensor.matmul(
                        sc[:, 0:512],
                        KT[m][0:64, i * 128 : (i + 1) * 128],
                        QT[m][0:64, sq * 512 : (sq + 1) * 512],
                        start=True, stop=True,
                    )
                    nc.tensor.matmul(
                        sc[:, 512:1024],
                        KT[m][64:128, i * 128 : (i + 1) * 128],
                        QT[m][64:128, sq * 512 : (sq + 1) * 512],
                        start=True, stop=True,
                        tile_position=(64, 0),
                    )
                    p = probs.tile([128, 1024], I16, name="pb", tag="pb")
                    if i in DVE_CHUNKS:
                        nc.vector.tensor_scalar(
                            p[:], sc[:], EXPA, EXPB, ALU.mult, ALU.add
                        )
                    else:
                        nc.scalar.activation(
                            p[:].bitcast(BF16), sc[:], EXP, scale=0.125
                        )
                    pb[i] = p

                def ctx(i):
                    pbf = pb.pop(i)[:].bitcast(BF16)
                    nc.tensor.matmul(
                        ctA[0:65, :], VH[i][:, 2 * m, 0:65], pbf[:, 0:512],
                        start=(i == 0), stop=(i == SKC - 1),
                    )
                    nc.tensor.matmul(
                        ctB[0:65, :], VH[i][:, 2 * m + 1, 0:65], pbf[:, 512:1024],
                        start=(i == 0), stop=(i == SKC - 1),
                    )

                for i in range(LAG):
                    scores(i)
                    if lead_pops > i and pe_fill:
                        pe_fill.pop(0)()
                for i in range(SKC):
                    if i in DVE_CHUNKS:
                        for _ in range(per_slot):
                            if pe_fill:
                                pe_fill.pop(0)()
                    if i in (4, 8) and norm_pieces:
                        norm_pieces.pop(0)()
                    ctx(i)
                    if i + LAG < SKC:
                        scores(i + LAG)
                while norm_pieces:
                    norm_pieces.pop(0)()

                # boundary evacuation: ctx rows on ACT (idle at the step
                # seam), denominator rows on DVE as f32r rounding copies
                # (the replicate matmul moves them at 1 cyc/row)
                ctsb = small.tile([64, 1024], F32, name="ctsb", tag="ctsb")
                nc.scalar.copy(ctsb[:, 0:512], ctA[0:64, :])
                nc.scalar.copy(ctsb[:, 512:1024], ctB[0:64, :])
                dn_t = small.tile([128, 1024], F32R, name="dnt", tag="dnt")
                with nc.allow_low_precision(
                    reason="f32r == 10-bit-mantissa denominators; 5e-4 rel"
                ):
                    nc.vector.tensor_copy(dn_t[64:65, 0:512], ctA[64:65, :])
                    nc.vector.tensor_copy(
                        dn_t[64:65, 512:1024], ctB[64:65, :]
                    )
                return ctsb, dn_t

            # ---- normalize pieces for the previous step, scheduled inside
            # the current one (REP as the first PE filler, the rest at
            # chunk slots 4 and 8) ----
            def norm_pieces_for(sq, m, ctsb, dn_t, pe_fill, mul_eng=None):
                mul_eng = mul_eng or nc.gpsimd
                rp = {}

                def rep_unit():
                    t = psc.tile([128, 1024], F32, name="sc", tag="sc")
                    nc.tensor.matmul(
                        t[0:64, 0:512], ones_t[64:65, 0:64],
                        dn_t[64:65, 0:512],
                        start=True, stop=True, tile_position=(64, 0),
                    )
                    nc.tensor.matmul(
                        t[0:64, 512:1024], ones_t[64:65, 0:64],
                        dn_t[64:65, 512:1024],
                        start=True, stop=True, tile_position=(64, 0),
                    )
                    rp["t"] = t

                rs = small.tile([64, 1024], F32, name="rs", tag="rs")

                def approx_piece():
                    t = rp["t"]
                    nc.vector.reciprocal_approx_fast(
                        rs[:, 0:512], t[0:64, 0:512]
                    )
                    nc.vector.reciprocal_approx_fast(
                        rs[:, 512:1024], t[0:64, 512:1024]
                    )

                def mul_piece():
                    mul_eng.tensor_mul(
                        ctxT[m][0:64, sq * 512 : (sq + 1) * 512],
                        ctsb[:, 0:512],
                        rs[:, 0:512],
                    )
                    stgB = small.tile([64, 512], BF16, name="stgB", tag="stgB")
                    mul_eng.tensor_mul(
                        stgB[:], ctsb[:, 512:1024], rs[:, 512:1024]
                    )
                    nc.sync.dma_start(
                        ctxT[m][64:128, sq * 512 : (sq + 1) * 512], stgB[:]
                    )

                pe_fill.insert(0, rep_unit)
                return [approx_piece, mul_piece]

            # ---- out-projection units: [128,1024] PSUM slot, 4 matmuls,
            # one DVE evacuation, one 4KB-row DMA ----
            def outproj_unit(sq128):
                def emit():
                    po = psc.tile([128, 1024], F32, name="sc", tag="sc")
                    for j in range(2):
                        for m in range(NP):
                            nc.tensor.matmul(
                                po[:, j * 512 : (j + 1) * 512],
                                ctxT[m][:, sq128 * 128 : (sq128 + 1) * 128],
                                wo_t[m][:, j * 512 : (j + 1) * 512],
                                start=(m == 0),
                                stop=(m == NP - 1),
                            )
                    ob = outsb.tile([128, 1024], F32, name="ob", tag="ob")
                    nc.vector.tensor_copy(ob[:], po[:])
                    nc.sync.dma_start(
                        outp[sq128 * 128 : (sq128 + 1) * 128, :], ob[:]
                    )

                return emit

            # ---- step loop: norm lags one step; fillers (REP, out-proj,
            # deferred Q-proj) pop inside attention at DVE-chunk slots ----
            pe_fill = [vproj_unit(i) for i in range(SKC)]
            pe_fill.extend(
                qproj_unit(nq, m) for nq in range(1, SQC) for m in range(NP)
            )
            pending = None
            for step in range(NP * SQC):
                sq, m = step // NP, step % NP
                norm_pieces = []
                psqm = None
                if pending is not None:
                    psqm = (pending[0], pending[1])
                    norm_pieces = norm_pieces_for(*pending, pe_fill)
                state = attn_step(
                    sq, m, pe_fill, norm_pieces,
                    lead_pops=3 if step == 0 else 0,
                    per_slot=3 if step == 0 else 1,
                )
                if psqm is not None and psqm[1] == NP - 1:
                    pe_fill.extend(
                        outproj_unit(s128)
                        for s128 in range(psqm[0] * 4, (psqm[0] + 1) * 4)
                    )
                pending = (sq, m, *state)

            # tail: final norm (muls on the faster, now-idle DVE) then the
            # remaining out-projection units
            tail_pieces = norm_pieces_for(*pending, pe_fill, mul_eng=nc.vector)
            pe_fill.insert(1, tail_pieces[0])
            pe_fill.insert(2, tail_pieces[1])
            pe_fill.extend(
                outproj_unit(s128)
                for s128 in range(pending[0] * 4, (pending[0] + 1) * 4)
            )
            for g in pe_fill:
                g()

    nc.compile()
    return nc


def get_program():
    if "nc" not in _CACHE:
        _CACHE["nc"] = _build_program()
    return _CACHE["nc"]


def make_in_maps(q, k, v, wq, bq, wk, bk, wv, bv, wo, bo):
    q, k, v = (np.asarray(x, np.float32) for x in (q, k, v))
    wq, wk, wv, wo = (np.asarray(x, np.float32) for x in (wq, wk, wv, wo))
    bq, bk, bv = (np.asarray(x, np.float32) for x in (bq, bk, bv))
    BF = ml_dtypes.bfloat16
    qT = [np.ascontiguousarray(q[b].T).astype(BF) for b in range(B)]
    kTt = [np.ascontiguousarray(k[b].T).astype(BF) for b in range(B)]
    vTt = [np.ascontiguousarray(v[b].T).astype(BF) for b in range(B)]
    in_maps = []
    def pack_w(w):
        # [D, GC] -> [128, KC*GC], partition-major: row p holds the GC-wide
        # slices of contraction chunks c at DRAM rows c*128+p (one 4KB line)
        return np.ascontiguousarray(
            w.reshape(KC, 128, GC).transpose(1, 0, 2).reshape(128, KC * GC)
        ).astype(BF)

    for c in range(NCORES):
        b, g = c // 4, c % 4
        sl = slice(g * GC, (g + 1) * GC)
        in_maps.append(
            {
                "qT": qT[b],
                "kT": kTt[b],
                "vT": vTt[b],
                "wqs": pack_w(wq[:, sl]),
                "wks": pack_w(wk[:, sl]),
                "wvs": pack_w(wv[:, sl]),
                "wos": np.ascontiguousarray(wo[sl, :]).astype(BF),
                "bqs": np.ascontiguousarray(bq[sl]).reshape(NP, 128, 1),
                "bks": np.ascontiguousarray(bk[sl]).reshape(NP, 128, 1),
                "bvs": np.ascontiguousarray(bv[sl]).reshape(1, GC),
            }
        )
    return in_maps


def combine_outputs(results, bo):
    out = np.zeros((B, S, D), np.float32)
    for c in range(NCORES):
        out[c // 4] += results[c]["outp"].astype(np.float32)
    out += np.asarray(bo, np.float32)
    return out


def kernel(q, k, v, wq, bq, wk, bk, wv, bv, wo, bo, trace=False):
    from concourse.bass_utils import run_bass_kernel_spmd

    nc = get_program()
    in_maps = make_in_maps(q, k, v, wq, bq, wk, bk, wv, bv, wo, bo)
    res = run_bass_kernel_spmd(nc, in_maps, list(range(NCORES)), trace=trace)
    out = combine_outputs(res.results, bo)
    if trace:
        _CACHE["last_result"] = res
    return out
